# revision 1
# baseline (speedup 1.0000x reference)
"""BiRNN (bidirectional Elman RNN) Trainium2 kernel.

Shapes (hardcoded from the problem spec):
  inputs [T=512, B=64, D=512] f32, W_xh_* [512,512], W_hh_* [512,512],
  b_h_* [512]; outputs [T, B, 2H] f32 plus final hidden states f_H, b_H.

Strategy: the sequence recurrence is serial, so batch-parallelism buys
nothing (the per-step matmul cost on the PE is weight-load-bound and
independent of B<=128).  Instead each direction runs wholly on ONE
NeuronCore (core 0 forward, core 1 backward, fully parallel), with all
state in transposed-packed layout h[128p, (4c x 64b)] so each step is:

  psum[128,256] = I.T @ xp_t                     (1 matmul, injects x-proj)
  psum[:, 64c:+64] += W_hh[kk,c].T @ h_{t-1}[kk] (16 matmuls, accumulate)
  h_t = tanh(psum)                               (1 ACT instruction)

The input projection x_t @ W_xh + b is computed ON THE SAME CORE as
filler matmuls (N=512) inside each step's tanh/semaphore window, and
lives in an SBUF ring — it never touches DRAM.  Everything is fp16 with
fp32 PSUM accumulation (the recurrence is strongly contractive, so fp16
step noise stays bounded; measured end-to-end error ~1e-3 relative).
"""

import numpy as np

T, B, D, H = 512, 64, 512, 512
GOUT = 4   # h steps per output DMA
RING = 4   # xp ring depth, in groups of 8 steps
NGRP = T // 8

_compiled = [None]
last_exec_time_ns = [None]


def _build():
    from concourse import bacc
    import concourse.tile as tile
    import concourse.mybir as mybir

    f16 = mybir.dt.float16
    f32 = mybir.dt.float32
    Tanh = mybir.ActivationFunctionType.Tanh

    nc = bacc.Bacc("TRN2", target_bir_lowering=False, debug=True)
    # xt[kk, p, t, b] = x[t, b, 128kk+p]
    xt = nc.dram_tensor("xt", [4, 128, T, 64], f16, kind="ExternalInput")
    whh = nc.dram_tensor("whh", [4, 128, 4, 128], f16, kind="ExternalInput")
    wxh = nc.dram_tensor("wxh", [4, 128, 4, 128], f16, kind="ExternalInput")
    bias = nc.dram_tensor("bias", [4, 128], f32, kind="ExternalInput")
    ident = nc.dram_tensor("ident", [128, 128], f16, kind="ExternalInput")
    ht = nc.dram_tensor("ht", [T, 4, 128, 64], f16, kind="ExternalOutput")

    with tile.TileContext(nc) as tc:
        with (
            tc.tile_pool(name="singles", bufs=1) as singles,
            tc.tile_pool(name="xts", bufs=3) as xts,
            tc.tile_pool(name="xpr", bufs=RING) as xpr,
            tc.tile_pool(name="hs", bufs=2) as hs,
            tc.tile_pool(name="ps", bufs=3, space="PSUM") as ps,
            tc.tile_pool(name="psx", bufs=2, space="PSUM") as psx,
        ):
            whh_sb = singles.tile([128, 4, 4, 128], f16)
            nc.sync.dma_start(whh_sb[:], whh.rearrange("kk p c m -> p kk c m"))
            wxh_sb = singles.tile([128, 4, 4, 128], f16)
            nc.sync.dma_start(wxh_sb[:], wxh.rearrange("kk p c m -> p kk c m"))
            bias_sb = singles.tile([128, 4], f32)
            nc.sync.dma_start(bias_sb[:], bias.rearrange("c p -> p c"))
            id_sb = singles.tile([128, 128], f16)
            nc.sync.dma_start(id_sb[:], ident[:])
            h0 = singles.tile([128, 4, 64], f16)
            nc.vector.memset(h0[:], 0.0)

            # ---- x-projection pipeline (filler work) ----
            xt_stage = {}

            def stage_group(g):
                if g >= NGRP or g in xt_stage:
                    return
                tl = xts.tile([128, 4, 512], f16, name="xts_t", tag="xts")
                nc.sync.dma_start(
                    tl[:],
                    xt.rearrange("kk p t b -> p kk (t b)")[:, :, 512 * g:512 * (g + 1)])
                xt_stage[g] = tl

            xp_ring = {}
            xp_psum = [None]
            xp_jobs = []

            def push_jobs(j):
                g, c = divmod(j, 4)
                if g >= NGRP:
                    return
                for kk in range(4):
                    xp_jobs.append((g, c, kk))

            def run_xp_mms(n):
                done = 0
                while done < n and xp_jobs:
                    g, c, kk = xp_jobs.pop(0)
                    if kk == 0:
                        stage_group(g)
                        xp_psum[0] = psx.tile([128, 512], f32, name="xpp", tag="psx")
                    nc.tensor.matmul(
                        xp_psum[0][:],
                        wxh_sb[:, kk, c, :],
                        xt_stage[g][:, kk, :],
                        start=(kk == 0), stop=(kk == 3),
                    )
                    if kk == 3:
                        r = g % RING
                        if c == 0:
                            xp_ring[r] = xpr.tile([128, 8, 4, 64], f16,
                                                  name="xpring", tag="xpr")
                        # evac + bias add in one DVE op
                        nc.vector.tensor_scalar_add(
                            xp_ring[r][:, :, c, :],
                            xp_psum[0].rearrange("p (t b) -> p t b", t=8),
                            bias_sb[:, c:c + 1],
                        )
                    done += 1

            for j in range(8):
                push_jobs(j)
            run_xp_mms(32)
            next_job = 8

            # ---- recurrence ----
            h_prev = h0
            h_stage = None
            for t in range(T):
                g, gi = divmod(t, 8)
                og = t % GOUT
                if og == 0:
                    h_stage = hs.tile([128, GOUT, 4, 64], f16, name="hst", tag="hs")

                psum = ps.tile([128, 256], f32, name="pst", tag="ps")
                nc.tensor.matmul(psum[:], id_sb[:],
                                 xp_ring[g % RING][:, gi, :, :],
                                 start=True, stop=False)
                for kk in range(4):
                    for c in range(4):
                        nc.tensor.matmul(
                            psum[:, 64 * c:64 * (c + 1)],
                            whh_sb[:, kk, c, :],
                            h_prev[:, kk, :],
                            start=False, stop=(kk == 3),
                            skip_group_check=True,
                        )
                h_t = h_stage[:, og, :, :]
                nc.scalar.activation(
                    out=h_t,
                    in_=psum.rearrange("p (c b) -> p c b", c=4),
                    func=Tanh,
                )
                if t % 2 == 0:
                    push_jobs(next_job)
                    next_job += 1
                run_xp_mms(2)

                if og == GOUT - 1:
                    nc.sync.dma_start(
                        ht.rearrange("t c p b -> p t c b")[:, t - og:t + 1, :, :],
                        h_stage[:])
                h_prev = h_t
    nc.compile()
    return nc


def _pack_w(w):
    # [K, M] f32 -> [4, 128, 4, 128] fp16 blocks w[kk, p, c, m]
    return np.ascontiguousarray(
        w.astype(np.float16).reshape(4, 128, 4, 128))


def _pack_xt(x16):
    # x fp16 [T, B, D] -> xt[kk, p, t, b]
    return np.ascontiguousarray(x16.transpose(2, 0, 1).reshape(4, 128, T, 64))


def _unpack_ht(ht):
    # [T, 4, 128, 64] fp16 -> [T, B, H] f32
    return ht.transpose(0, 3, 1, 2).reshape(T, B, H).astype(np.float32)


def run(inputs, W_xh_f, W_hh_f, b_h_f, W_xh_b, W_hh_b, b_h_b, trace=False):
    from concourse.bass_utils import run_bass_kernel_spmd

    if _compiled[0] is None:
        _compiled[0] = _build()
    nc = _compiled[0]

    x16 = np.asarray(inputs, dtype=np.float32).astype(np.float16)
    ident = np.eye(128, dtype=np.float16)

    in_f = {
        "xt": _pack_xt(x16),
        "whh": _pack_w(np.asarray(W_hh_f, dtype=np.float32)),
        "wxh": _pack_w(np.asarray(W_xh_f, dtype=np.float32)),
        "bias": np.ascontiguousarray(
            np.asarray(b_h_f, dtype=np.float32).reshape(4, 128)),
        "ident": ident,
    }
    in_b = {
        "xt": _pack_xt(x16[::-1]),
        "whh": _pack_w(np.asarray(W_hh_b, dtype=np.float32)),
        "wxh": _pack_w(np.asarray(W_xh_b, dtype=np.float32)),
        "bias": np.ascontiguousarray(
            np.asarray(b_h_b, dtype=np.float32).reshape(4, 128)),
        "ident": ident,
    }

    res = run_bass_kernel_spmd(nc, [in_f, in_b], [0, 1], trace=trace)
    last_exec_time_ns[0] = res.exec_time_ns

    f_outs = _unpack_ht(res.results[0]["ht"])          # [T, B, H]
    b_outs_rev = _unpack_ht(res.results[1]["ht"])      # reversed time

    outputs = np.empty((T, B, 2 * H), dtype=np.float32)
    outputs[:, :, :H] = f_outs
    outputs[:, :, H:] = b_outs_rev[::-1]
    f_H = f_outs[T - 1].copy()
    b_H = b_outs_rev[T - 1].copy()
    return outputs, f_H, b_H


def kernel(inputs, W_xh_f, W_hh_f, b_h_f, W_xh_b, W_hh_b, b_h_b):
    return run(inputs, W_xh_f, W_hh_f, b_h_f, W_xh_b, W_hh_b, b_h_b)


# revision 4
# speedup vs baseline: 1.1451x; 1.1451x over previous
"""BiRNN (bidirectional Elman RNN) Trainium2 kernel.

Shapes (hardcoded from the problem spec):
  inputs [T=512, B=64, D=512] f32, W_xh_* [512,512], W_hh_* [512,512],
  b_h_* [512]; outputs [T, B, 2H] f32 plus final hidden states f_H, b_H.

Strategy: the sequence recurrence is serial, so batch-parallelism buys
nothing (the per-step matmul cost on the PE is weight-load-bound and
independent of B<=128).  Instead each direction runs wholly on ONE
NeuronCore (core 0 forward, core 1 backward, fully parallel), with all
state in transposed-packed layout h[128p, (4c x 64b)] so each step is:

  psum[128,256] = I.T @ xp_t                     (1 matmul, injects x-proj)
  psum[:, 64c:+64] += W_hh[kk,c].T @ h_{t-1}[kk] (16 matmuls, accumulate)
  h_t = tanh(psum)                               (1 ACT instruction)

The input projection x_t @ W_xh + b is computed ON THE SAME CORE as
filler matmuls (N=512) inside each step's tanh/semaphore window, and
lives in an SBUF ring — it never touches DRAM.  Everything is fp16 with
fp32 PSUM accumulation (the recurrence is strongly contractive, so fp16
step noise stays bounded; measured end-to-end error ~1e-3 relative).
"""

import numpy as np

T, B, D, H = 512, 64, 512, 512
GOUT = 4   # h steps per output DMA
RING = 4   # xp ring depth, in groups of 8 steps
NGRP = T // 8

_compiled = [None]
last_exec_time_ns = [None]


def _build():
    from concourse import bacc
    import concourse.tile as tile
    import concourse.mybir as mybir

    f16 = mybir.dt.float16
    f32 = mybir.dt.float32
    Tanh = mybir.ActivationFunctionType.Tanh

    nc = bacc.Bacc("TRN2", target_bir_lowering=False, debug=True)
    # xt[kk, p, t, b] = x[t, b, 128kk+p]
    xt = nc.dram_tensor("xt", [4, 128, T, 64], f16, kind="ExternalInput")
    whh = nc.dram_tensor("whh", [4, 128, 4, 128], f16, kind="ExternalInput")
    wxh = nc.dram_tensor("wxh", [4, 128, 4, 128], f16, kind="ExternalInput")
    bias = nc.dram_tensor("bias", [4, 128], f32, kind="ExternalInput")
    ident = nc.dram_tensor("ident", [128, 128], f16, kind="ExternalInput")
    ht = nc.dram_tensor("ht", [T, 4, 128, 64], f16, kind="ExternalOutput")

    with tile.TileContext(nc) as tc:
        with (
            tc.tile_pool(name="singles", bufs=1) as singles,
            tc.tile_pool(name="xts", bufs=3) as xts,
            tc.tile_pool(name="xpr", bufs=RING) as xpr,
            tc.tile_pool(name="hs", bufs=2) as hs,
            tc.tile_pool(name="ps", bufs=2, space="PSUM") as ps,
            tc.tile_pool(name="psx", bufs=2, space="PSUM") as psx,
        ):
            whh_sb = singles.tile([128, 4, 4, 128], f16)
            nc.sync.dma_start(whh_sb[:], whh.rearrange("kk p c m -> p kk c m"))
            wxh_sb = singles.tile([128, 4, 4, 128], f16)
            nc.sync.dma_start(wxh_sb[:], wxh.rearrange("kk p c m -> p kk c m"))
            bias_sb = singles.tile([128, 4], f32)
            nc.sync.dma_start(bias_sb[:], bias.rearrange("c p -> p c"))
            id_sb = singles.tile([128, 128], f16)
            nc.sync.dma_start(id_sb[:], ident[:])
            h0 = singles.tile([128, 4, 64], f16)
            nc.vector.memset(h0[:], 0.0)

            # ---- x-projection pipeline (filler work) ----
            xt_stage = {}

            def stage_group(g):
                if g >= NGRP or g in xt_stage:
                    return
                tl = xts.tile([128, 4, 512], f16, name="xts_t", tag="xts")
                nc.sync.dma_start(
                    tl[:],
                    xt.rearrange("kk p t b -> p kk (t b)")[:, :, 512 * g:512 * (g + 1)])
                xt_stage[g] = tl

            xp_ring = {}
            xp_psum = [None]
            xp_jobs = []

            def push_jobs(j):
                g, c = divmod(j, 4)
                if g >= NGRP:
                    return
                for kk in range(4):
                    xp_jobs.append((g, c, kk))

            def run_xp_mms(n, allow_dummy=False):
                done = 0
                while done < n and xp_jobs:
                    g, c, kk = xp_jobs.pop(0)
                    if kk == 0:
                        stage_group(g)
                        xp_psum[0] = psx.tile([128, 512], f32, name="xpp", tag="psx")
                    nc.tensor.matmul(
                        xp_psum[0][:],
                        wxh_sb[:, kk, c, :],
                        xt_stage[g][:, kk, :],
                        start=(kk == 0), stop=(kk == 3),
                    )
                    if kk == 3:
                        r = g % RING
                        if c == 0:
                            xp_ring[r] = xpr.tile([128, 8, 4, 64], f16,
                                                  name="xpring", tag="xpr")
                        # evac + bias add in one DVE op
                        nc.vector.tensor_scalar_add(
                            xp_ring[r][:, :, c, :],
                            xp_psum[0].rearrange("p (t b) -> p t b", t=8),
                            bias_sb[:, c:c + 1],
                        )
                    done += 1
                if done < n and allow_dummy:
                    # keep the PE busy so the HAM clock gate stays at 8/8
                    # through the tail steps (real x-proj work is exhausted)
                    scratch = psx.tile([128, 512], f32, name="xpp", tag="psx")
                    for i in range(n - done):
                        nc.tensor.matmul(
                            scratch[:],
                            whh_sb[:, 0, 0, :],
                            wxh_sb[:, 0, :, :].rearrange("p c m -> p (c m)"),
                            start=(i == 0), stop=(i == n - done - 1),
                        )

            for j in range(8):
                push_jobs(j)
            run_xp_mms(32)
            next_job = 8

            # ---- recurrence ----
            h_prev = h0
            h_stage = None
            for t in range(T):
                g, gi = divmod(t, 8)
                og = t % GOUT
                if og == 0:
                    h_stage = hs.tile([128, GOUT, 4, 64], f16, name="hst", tag="hs")

                # two PSUM banks so the c0-2 tanh (bank A) can run while the
                # PE still accumulates the c3 column (bank B) — an uneven
                # (3+1) split that takes most of the tanh latency off the
                # serial chain without saturating the Scalar engine
                psA = ps.tile([128, 192], f32, name="psA", tag="psA")
                psB = ps.tile([128, 64], f32, name="psB", tag="psB")
                xpv = xp_ring[g % RING][:, gi, :, :].rearrange("p c b -> p (c b)")
                nc.tensor.matmul(psA[:], id_sb[:], xpv[:, 0:192],
                                 start=True, stop=False)
                nc.tensor.matmul(psB[:], id_sb[:], xpv[:, 192:256],
                                 start=True, stop=False)
                for kk in range(4):
                    for c in range(3):
                        nc.tensor.matmul(
                            psA[:, 64 * c:64 * (c + 1)],
                            whh_sb[:, kk, c, :],
                            h_prev[:, kk, :],
                            start=False, stop=(kk == 3),
                            skip_group_check=True,
                        )
                h_t = h_stage[:, og, :, :]
                nc.scalar.activation(
                    out=h_t[:, 0:3, :],
                    in_=psA.rearrange("p (c b) -> p c b", c=3),
                    func=Tanh,
                )
                for kk in range(4):
                    nc.tensor.matmul(
                        psB[:],
                        whh_sb[:, kk, 3, :],
                        h_prev[:, kk, :],
                        start=False, stop=(kk == 3),
                        skip_group_check=True,
                    )
                nc.scalar.activation(
                    out=h_t[:, 3:4, :],
                    in_=psB.rearrange("p (c b) -> p c b", c=1),
                    func=Tanh,
                )
                if t % 2 == 0:
                    push_jobs(next_job)
                    next_job += 1
                run_xp_mms(2, allow_dummy=(t >= 448))

                if og == GOUT - 1:
                    nc.sync.dma_start(
                        ht.rearrange("t c p b -> p t c b")[:, t - og:t + 1, :, :],
                        h_stage[:])
                h_prev = h_t
    nc.compile()
    return nc


def _pack_w(w):
    # [K, M] f32 -> [4, 128, 4, 128] fp16 blocks w[kk, p, c, m]
    return np.ascontiguousarray(
        w.astype(np.float16).reshape(4, 128, 4, 128))


def _pack_xt(x16):
    # x fp16 [T, B, D] -> xt[kk, p, t, b]
    return np.ascontiguousarray(x16.transpose(2, 0, 1).reshape(4, 128, T, 64))


def _unpack_ht(ht):
    # [T, 4, 128, 64] fp16 -> [T, B, H] f32
    return ht.transpose(0, 3, 1, 2).reshape(T, B, H).astype(np.float32)


def run(inputs, W_xh_f, W_hh_f, b_h_f, W_xh_b, W_hh_b, b_h_b, trace=False):
    from concourse.bass_utils import run_bass_kernel_spmd

    if _compiled[0] is None:
        _compiled[0] = _build()
    nc = _compiled[0]

    x16 = np.asarray(inputs, dtype=np.float32).astype(np.float16)
    ident = np.eye(128, dtype=np.float16)

    in_f = {
        "xt": _pack_xt(x16),
        "whh": _pack_w(np.asarray(W_hh_f, dtype=np.float32)),
        "wxh": _pack_w(np.asarray(W_xh_f, dtype=np.float32)),
        "bias": np.ascontiguousarray(
            np.asarray(b_h_f, dtype=np.float32).reshape(4, 128)),
        "ident": ident,
    }
    in_b = {
        "xt": _pack_xt(x16[::-1]),
        "whh": _pack_w(np.asarray(W_hh_b, dtype=np.float32)),
        "wxh": _pack_w(np.asarray(W_xh_b, dtype=np.float32)),
        "bias": np.ascontiguousarray(
            np.asarray(b_h_b, dtype=np.float32).reshape(4, 128)),
        "ident": ident,
    }

    res = run_bass_kernel_spmd(nc, [in_f, in_b], [0, 1], trace=trace)
    last_exec_time_ns[0] = res.exec_time_ns

    f_outs = _unpack_ht(res.results[0]["ht"])          # [T, B, H]
    b_outs_rev = _unpack_ht(res.results[1]["ht"])      # reversed time

    outputs = np.empty((T, B, 2 * H), dtype=np.float32)
    outputs[:, :, :H] = f_outs
    outputs[:, :, H:] = b_outs_rev[::-1]
    f_H = f_outs[T - 1].copy()
    b_H = b_outs_rev[T - 1].copy()
    return outputs, f_H, b_H


def kernel(inputs, W_xh_f, W_hh_f, b_h_f, W_xh_b, W_hh_b, b_h_b):
    return run(inputs, W_xh_f, W_hh_f, b_h_f, W_xh_b, W_hh_b, b_h_b)


# revision 6
# speedup vs baseline: 1.1807x; 1.0311x over previous
"""BiRNN (bidirectional Elman RNN) Trainium2 kernel.

Shapes (hardcoded from the problem spec):
  inputs [T=512, B=64, D=512] f32, W_xh_* [512,512], W_hh_* [512,512],
  b_h_* [512]; outputs [T, B, 2H] f32 plus final hidden states f_H, b_H.

Strategy: the sequence recurrence is serial, so batch-parallelism buys
nothing (the per-step matmul cost on the PE is weight-load-bound and
independent of B<=128).  Instead each direction runs wholly on ONE
NeuronCore (core 0 forward, core 1 backward, fully parallel), with all
state in transposed-packed layout h[128p, (4c x 64b)] so each step is:

  psum[128,256] = I.T @ xp_t                     (1 matmul, injects x-proj)
  psum[:, 64c:+64] += W_hh[kk,c].T @ h_{t-1}[kk] (16 matmuls, accumulate)
  h_t = tanh(psum)                               (1 ACT instruction)

The input projection x_t @ W_xh + b is computed ON THE SAME CORE as
filler matmuls (N=512) inside each step's tanh/semaphore window, and
lives in an SBUF ring — it never touches DRAM.  Everything is fp16 with
fp32 PSUM accumulation (the recurrence is strongly contractive, so fp16
step noise stays bounded; measured end-to-end error ~1e-3 relative).
"""

import numpy as np

T, B, D, H = 512, 64, 512, 512
GOUT = 4   # h steps per output DMA
RING = 5   # xp ring depth, in groups of 8 steps
NGRP = T // 8

_compiled = [None]
last_exec_time_ns = [None]


def _build():
    from concourse import bacc
    import concourse.tile as tile
    import concourse.mybir as mybir

    f16 = mybir.dt.float16
    f32 = mybir.dt.float32
    Tanh = mybir.ActivationFunctionType.Tanh

    nc = bacc.Bacc("TRN2", target_bir_lowering=False, debug=True)
    # xt[kk, p, t, b] = x[t, b, 128kk+p]
    xt = nc.dram_tensor("xt", [4, 128, T, 64], f16, kind="ExternalInput")
    whh = nc.dram_tensor("whh", [4, 128, 4, 128], f16, kind="ExternalInput")
    wxh = nc.dram_tensor("wxh", [4, 128, 4, 128], f16, kind="ExternalInput")
    bias = nc.dram_tensor("bias", [4, 128], f32, kind="ExternalInput")
    ident = nc.dram_tensor("ident", [128, 128], f16, kind="ExternalInput")
    ht = nc.dram_tensor("ht", [T, 4, 128, 64], f16, kind="ExternalOutput")

    with tile.TileContext(nc) as tc:
        with (
            tc.tile_pool(name="singles", bufs=1) as singles,
            tc.tile_pool(name="xts", bufs=4) as xts,
            tc.tile_pool(name="xpr", bufs=RING) as xpr,
            tc.tile_pool(name="hs", bufs=4) as hs,
            tc.tile_pool(name="ps", bufs=2, space="PSUM") as ps,
            tc.tile_pool(name="psx", bufs=3, space="PSUM") as psx,
        ):
            whh_sb = singles.tile([128, 4, 4, 128], f16)
            nc.sync.dma_start(whh_sb[:], whh.rearrange("kk p c m -> p kk c m"))
            wxh_sb = singles.tile([128, 4, 4, 128], f16)
            nc.sync.dma_start(wxh_sb[:], wxh.rearrange("kk p c m -> p kk c m"))
            bias_sb = singles.tile([128, 4], f32)
            nc.sync.dma_start(bias_sb[:], bias.rearrange("c p -> p c"))
            id_sb = singles.tile([128, 128], f16)
            nc.sync.dma_start(id_sb[:], ident[:])
            h0 = singles.tile([128, 4, 64], f16)
            nc.vector.memset(h0[:], 0.0)

            # ---- x-projection pipeline (filler work) ----
            xt_stage = {}

            def stage_group(g):
                if g >= NGRP or g in xt_stage:
                    return
                tl = xts.tile([128, 4, 512], f16, name="xts_t", tag="xts")
                nc.sync.dma_start(
                    tl[:],
                    xt.rearrange("kk p t b -> p kk (t b)")[:, :, 512 * g:512 * (g + 1)])
                xt_stage[g] = tl

            xp_ring = {}
            xp_psum = [None]
            xp_jobs = []

            def push_jobs(j):
                g, c = divmod(j, 4)
                if g >= NGRP:
                    return
                for kk in range(4):
                    xp_jobs.append((g, c, kk))

            def run_xp_mms(n, allow_dummy=False):
                done = 0
                while done < n and xp_jobs:
                    g, c, kk = xp_jobs.pop(0)
                    if kk == 0:
                        stage_group(g)
                        xp_psum[0] = psx.tile([128, 512], f32, name="xpp", tag="psx")
                    nc.tensor.matmul(
                        xp_psum[0][:],
                        wxh_sb[:, kk, c, :],
                        xt_stage[g][:, kk, :],
                        start=(kk == 0), stop=(kk == 3),
                    )
                    if kk == 3:
                        r = g % RING
                        if c == 0:
                            xp_ring[r] = xpr.tile([128, 8, 4, 64], f16,
                                                  name="xpring", tag="xpr")
                        # evac + bias add in one DVE op
                        nc.vector.tensor_scalar_add(
                            xp_ring[r][:, :, c, :],
                            xp_psum[0].rearrange("p (t b) -> p t b", t=8),
                            bias_sb[:, c:c + 1],
                        )
                    done += 1
                if done < n and allow_dummy:
                    # keep the PE busy so the HAM clock gate stays at 8/8
                    # through the tail steps (real x-proj work is exhausted)
                    scratch = psx.tile([128, 512], f32, name="xpp", tag="psx")
                    for i in range(n - done):
                        nc.tensor.matmul(
                            scratch[:],
                            whh_sb[:, 0, 0, :],
                            wxh_sb[:, 0, :, :].rearrange("p c m -> p (c m)"),
                            start=(i == 0), stop=(i == n - done - 1),
                        )

            for j in range(6):
                push_jobs(j)
            run_xp_mms(16)
            next_job = 6

            # ---- recurrence ----
            h_prev = h0
            h_stage = None
            for t in range(T):
                g, gi = divmod(t, 8)
                og = t % GOUT
                if og == 0:
                    h_stage = hs.tile([128, GOUT, 4, 64], f16, name="hst", tag="hs")

                # two PSUM banks so the c0-2 tanh (bank A) can run while the
                # PE still accumulates the c3 column (bank B) — an uneven
                # (3+1) split that takes most of the tanh latency off the
                # serial chain without saturating the Scalar engine
                psA = ps.tile([128, 192], f32, name="psA", tag="psA")
                psB = ps.tile([128, 64], f32, name="psB", tag="psB")
                xpv = xp_ring[g % RING][:, gi, :, :].rearrange("p c b -> p (c b)")
                nc.tensor.matmul(psA[:], id_sb[:], xpv[:, 0:192],
                                 start=True, stop=False)
                nc.tensor.matmul(psB[:], id_sb[:], xpv[:, 192:256],
                                 start=True, stop=False)
                for kk in range(4):
                    for c in range(3):
                        nc.tensor.matmul(
                            psA[:, 64 * c:64 * (c + 1)],
                            whh_sb[:, kk, c, :],
                            h_prev[:, kk, :],
                            start=False, stop=(kk == 3),
                            skip_group_check=True,
                        )
                h_t = h_stage[:, og, :, :]
                nc.scalar.activation(
                    out=h_t[:, 0:3, :],
                    in_=psA.rearrange("p (c b) -> p c b", c=3),
                    func=Tanh,
                )
                for kk in range(4):
                    nc.tensor.matmul(
                        psB[:],
                        whh_sb[:, kk, 3, :],
                        h_prev[:, kk, :],
                        start=False, stop=(kk == 3),
                        skip_group_check=True,
                    )
                nc.scalar.activation(
                    out=h_t[:, 3:4, :],
                    in_=psB.rearrange("p (c b) -> p c b", c=1),
                    func=Tanh,
                )
                if t % 2 == 0:
                    push_jobs(next_job)
                    next_job += 1
                run_xp_mms(2, allow_dummy=(t >= 448))

                if og == GOUT - 1:
                    nc.sync.dma_start(
                        ht.rearrange("t c p b -> p t c b")[:, t - og:t + 1, :, :],
                        h_stage[:])
                h_prev = h_t
    nc.compile()
    return nc


def _pack_w(w):
    # [K, M] f32 -> [4, 128, 4, 128] fp16 blocks w[kk, p, c, m]
    return np.ascontiguousarray(
        w.astype(np.float16).reshape(4, 128, 4, 128))


def _pack_xt(x16):
    # x fp16 [T, B, D] -> xt[kk, p, t, b]
    return np.ascontiguousarray(x16.transpose(2, 0, 1).reshape(4, 128, T, 64))


def _unpack_ht(ht):
    # [T, 4, 128, 64] fp16 -> [T, B, H] f32
    return ht.transpose(0, 3, 1, 2).reshape(T, B, H).astype(np.float32)


def run(inputs, W_xh_f, W_hh_f, b_h_f, W_xh_b, W_hh_b, b_h_b, trace=False):
    from concourse.bass_utils import run_bass_kernel_spmd

    if _compiled[0] is None:
        _compiled[0] = _build()
    nc = _compiled[0]

    x16 = np.asarray(inputs, dtype=np.float32).astype(np.float16)
    ident = np.eye(128, dtype=np.float16)

    in_f = {
        "xt": _pack_xt(x16),
        "whh": _pack_w(np.asarray(W_hh_f, dtype=np.float32)),
        "wxh": _pack_w(np.asarray(W_xh_f, dtype=np.float32)),
        "bias": np.ascontiguousarray(
            np.asarray(b_h_f, dtype=np.float32).reshape(4, 128)),
        "ident": ident,
    }
    in_b = {
        "xt": _pack_xt(x16[::-1]),
        "whh": _pack_w(np.asarray(W_hh_b, dtype=np.float32)),
        "wxh": _pack_w(np.asarray(W_xh_b, dtype=np.float32)),
        "bias": np.ascontiguousarray(
            np.asarray(b_h_b, dtype=np.float32).reshape(4, 128)),
        "ident": ident,
    }

    res = run_bass_kernel_spmd(nc, [in_f, in_b], [0, 1], trace=trace)
    last_exec_time_ns[0] = res.exec_time_ns

    f_outs = _unpack_ht(res.results[0]["ht"])          # [T, B, H]
    b_outs_rev = _unpack_ht(res.results[1]["ht"])      # reversed time

    outputs = np.empty((T, B, 2 * H), dtype=np.float32)
    outputs[:, :, :H] = f_outs
    outputs[:, :, H:] = b_outs_rev[::-1]
    f_H = f_outs[T - 1].copy()
    b_H = b_outs_rev[T - 1].copy()
    return outputs, f_H, b_H


def kernel(inputs, W_xh_f, W_hh_f, b_h_f, W_xh_b, W_hh_b, b_h_b):
    return run(inputs, W_xh_f, W_hh_f, b_h_f, W_xh_b, W_hh_b, b_h_b)
